# revision 24
# baseline (speedup 1.0000x reference)
"""Trainium2 Bass kernel for BGEM3 sparse-embedding head (segment_reduce).

Computes, for inputs hidden_state [B,S,H], input_ids [B,S], W_sparse [1,H],
b_sparse [1]:
    tw = relu(hidden_state @ W_sparse[0] + b_sparse[0])          # [B,S]
    out = zeros([B,V]); out.at[b, ids].max(tw)  (jax scatter-max, which on
    this stack sums duplicate indices); out[:, 0:4] = 0
Sharding: data-parallel over batch across 8 NeuronCores (4 rows per core).

Per core, per batch row (8 column-tiles of 128 tokens; token s = 128j + p):
  1. matvec: fused mult + add-reduce (DVE scalar_tensor_tensor + accum)
     against a W tile broadcast via PE (ones outer product); relu(x+b) on
     ACT; special ids (< 4) masked to 0.
  2. eq[p,j,q] = (id[128j+p] == id[128j+q]): ids transposed per column on PE
     (broadcast-transpose into PSUM), one batched [128,8,128] DVE pass.
     gsum[:, j] = eq_j @ twm[:, j] (one PE matmul per column): every token
     gets its within-column duplicate-group sum.
  3. Column j of row r scatters (plain writes, 128 offsets = one per
     partition, a hardware limit) into its OWN table section at offset
     j*VPAD. Within a column, duplicate offsets all carry the identical
     group sum, so collisions are benign; across columns the sections are
     disjoint, so no ordering constraints exist anywhere -> the 32 scatters
     stream back-to-back on GpSimd. The host sums the 8 sections per row
     while unsharding (the standard gather for a sum-sharded output).
Output tables rely on the runtime's zero-donated output buffers
(run_bass_via_pjrt donates np.zeros as the ExternalOutput backing).
"""

import numpy as np

B, S, H, V = 32, 1024, 1024, 250002
N_CORES = 8
B_LOC = B // N_CORES          # 4 batch rows per core
VPAD = 128 * 1954             # 250112 >= V, divisible by 128
N_STILE = S // 128            # 8 column-tiles per row

_compiled = {}


def _build(b_val: float):
    import concourse.bass as bass
    import concourse.tile as tile
    from concourse import bacc, mybir
    from concourse.masks import make_identity

    f32 = mybir.dt.float32
    i32 = mybir.dt.int32
    Alu = mybir.AluOpType

    nc = bacc.Bacc("TRN2", target_bir_lowering=False, debug=False)

    hs = nc.dram_tensor("hs", [B_LOC, S, H], f32, kind="ExternalInput")
    ids = nc.dram_tensor("ids", [B_LOC, S], i32, kind="ExternalInput")
    w = nc.dram_tensor("w", [1, H], f32, kind="ExternalInput")
    tables = [
        [
            nc.dram_tensor(f"t{r}_{j}", [VPAD, 1], f32, kind="ExternalOutput")
            for j in range(N_STILE)
        ]
        for r in range(B_LOC)
    ]

    with tile.TileContext(nc) as tc:
        with (
            tc.tile_pool(name="const", bufs=1) as const_pool,
            tc.tile_pool(name="h", bufs=6) as h_pool,
            tc.tile_pool(name="eq", bufs=2) as eq_pool,
            tc.tile_pool(name="sc", bufs=2) as sc_pool,
            tc.tile_pool(name="sm", bufs=2) as sm_pool,
            tc.tile_pool(name="ps", bufs=2, space="PSUM") as ps_pool,
        ):
            # ---- W broadcast to 128 partitions via PE outer product ----
            ones1 = const_pool.tile([1, 128], f32)
            nc.vector.memset(ones1[:], 1.0)
            w_row = const_pool.tile([1, H], f32)
            nc.sync.dma_start(w_row[:], w[0:1, :])
            wb_ps = ps_pool.tile([128, H], f32, tag="wb_ps", bufs=1)
            for half in range(2):
                sl = slice(512 * half, 512 * (half + 1))
                nc.tensor.matmul(
                    out=wb_ps[:, sl],
                    lhsT=ones1[:],
                    rhs=w_row[:, sl],
                    start=True,
                    stop=True,
                )
            w_bc = const_pool.tile([128, H], f32)
            nc.scalar.copy(w_bc[:], wb_ps[:])

            ident = const_pool.tile([128, 128], f32)
            make_identity(nc, ident[:])

            # ---- all rows' ids in one tile: (p, r, j) = ids[r, 128j+p] ----
            idc_all = sm_pool.tile([128, B_LOC, N_STILE], i32, bufs=1)
            idc_f_all = sm_pool.tile([128, B_LOC, N_STILE], f32, bufs=1)
            for r in range(B_LOC):
                nc.sync.dma_start(
                    idc_all[:, r].rearrange("p (blk k) -> p blk k", k=4),
                    ids[r, :].rearrange("(blk p k) -> p blk k", p=128, k=4),
                )
                nc.scalar.copy(idc_f_all[:, r], idc_all[:, r])

            c3 = [128, N_STILE, 128]
            HALF = N_STILE // 2
            for r in range(B_LOC):
                idc_fr = idc_f_all[:, r]

                def emit_eq(r_, idc_fr_):
                    idT_ps = ps_pool.tile(
                        [128, S], f32, tag="idT_ps", bufs=1, name=f"idT{r_}"
                    )
                    for j in range(N_STILE):
                        nc.tensor.transpose(
                            out=idT_ps[:, 128 * j : 128 * (j + 1)],
                            in_=idc_fr_[:, j : j + 1].to_broadcast([128, 128]),
                            identity=ident[:],
                        )
                    eq8_ = eq_pool.tile(
                        [128, S], f32, tag="eq8", name=f"eq8_{r_}"
                    )
                    nc.vector.tensor_tensor(
                        out=eq8_[:].rearrange("p (j q) -> p j q", j=N_STILE),
                        in0=idc_fr_[:, :, None].to_broadcast(c3),
                        in1=idT_ps[:].rearrange("p (j q) -> p j q", j=N_STILE),
                        op=Alu.is_equal,
                    )
                    return eq8_

                # id-only work: for row 0 it would delay the first matvec
                # (head-of-line on the DVE queue), so emit it after the first
                # half-row instead; later rows emit it up front so the
                # row-tail chain never waits on it.
                eq8 = None
                if r > 0:
                    eq8 = emit_eq(r, idc_fr)

                # ---- matvec + per-half relu/mask/gsum/scatter chains ----
                twraw = sm_pool.tile([128, N_STILE], f32, tag="twraw")
                twrelu = sm_pool.tile([128, N_STILE], f32, tag="twrelu")
                twm = sm_pool.tile([128, N_STILE], f32, tag="twm")
                gsum_ps = ps_pool.tile([128, N_STILE], f32, tag="gsum")
                gsum = sm_pool.tile([128, N_STILE], f32, tag="gsumsb")
                for blk in range(N_STILE // 4):
                    ht = h_pool.tile([128, 4, H], f32, tag="h", bufs=4)
                    nc.sync.dma_start(
                        ht[:],
                        hs[r, 512 * blk : 512 * (blk + 1), :].rearrange(
                            "(p k) h -> p k h", p=128
                        ),
                    )
                    for k in range(4):
                        j = 4 * blk + k
                        prod = sc_pool.tile([128, H], f32, tag="prod")
                        nc.vector.scalar_tensor_tensor(
                            out=prod[:],
                            in0=ht[:, k],
                            scalar=1.0,
                            in1=w_bc[:],
                            op0=Alu.mult,
                            op1=Alu.mult,
                            accum_out=twraw[:, j : j + 1],
                        )
                    if r == 0 and blk == 0:
                        eq8 = emit_eq(r, idc_fr)
                    j = 4 * blk + 3
                    if j % HALF != HALF - 1:
                        continue
                    # half-row [j-HALF+1 .. j] is complete: finish it
                    h0 = j - HALF + 1
                    sl = slice(h0, j + 1)
                    nc.scalar.activation(
                        twrelu[:, sl],
                        twraw[:, sl],
                        mybir.ActivationFunctionType.Relu,
                        bias=float(b_val),
                    )
                    nc.vector.scalar_tensor_tensor(
                        out=twm[:, sl],
                        in0=idc_fr[:, sl],
                        scalar=4.0,
                        in1=twrelu[:, sl],
                        op0=Alu.is_ge,
                        op1=Alu.mult,
                    )
                    for jj in range(h0, j + 1):
                        nc.tensor.matmul(
                            out=gsum_ps[:, jj : jj + 1],
                            lhsT=eq8[:, 128 * jj : 128 * (jj + 1)],
                            rhs=twm[:, jj : jj + 1],
                            start=True,
                            stop=True,
                        )
                        nc.scalar.copy(
                            gsum[:, jj : jj + 1], gsum_ps[:, jj : jj + 1]
                        )
                        nc.gpsimd.indirect_dma_start(
                            out=tables[r][jj][:],
                            out_offset=bass.IndirectOffsetOnAxis(
                                ap=idc_all[:, r, jj : jj + 1], axis=0
                            ),
                            in_=gsum[:, jj : jj + 1],
                            in_offset=None,
                        )

    nc.compile()
    return nc


def _get_nc(b_val: float):
    key = float(b_val)
    if key not in _compiled:
        _compiled[key] = _build(key)
    return _compiled[key]


def kernel(hidden_state, input_ids, W_sparse, b_sparse):
    from concourse.bass_utils import run_bass_kernel_spmd

    hidden_state = np.ascontiguousarray(np.asarray(hidden_state, dtype=np.float32))
    input_ids = np.ascontiguousarray(np.asarray(input_ids, dtype=np.int32))
    W_sparse = np.ascontiguousarray(np.asarray(W_sparse, dtype=np.float32))
    b_val = float(np.asarray(b_sparse).reshape(-1)[0])

    nc = _get_nc(b_val)

    in_maps = []
    for c in range(N_CORES):
        sl = slice(c * B_LOC, (c + 1) * B_LOC)
        in_maps.append(
            {"hs": hidden_state[sl], "ids": input_ids[sl], "w": W_sparse}
        )

    res = run_bass_kernel_spmd(nc, in_maps, list(range(N_CORES)))

    out = np.empty((B, V), dtype=np.float32)
    for c in range(N_CORES):
        for r in range(B_LOC):
            acc = res.results[c][f"t{r}_0"][:V, 0].copy()
            for j in range(1, N_STILE):
                acc += res.results[c][f"t{r}_{j}"][:V, 0]
            out[c * B_LOC + r] = acc
    return out


# revision 25
# speedup vs baseline: 1.0989x; 1.0989x over previous
"""Trainium2 Bass kernel for BGEM3 sparse-embedding head (segment_reduce).

Computes, for inputs hidden_state [B,S,H], input_ids [B,S], W_sparse [1,H],
b_sparse [1]:
    tw = relu(hidden_state @ W_sparse[0] + b_sparse[0])          # [B,S]
    out = zeros([B,V]); out.at[b, ids].max(tw)  (jax scatter-max, which on
    this stack sums duplicate indices); out[:, 0:4] = 0
Sharding: data-parallel over batch across 8 NeuronCores (4 rows per core).

Per core, per batch row (8 column-tiles of 128 tokens; token s = 128j + p):
  1. matvec: fused mult + add-reduce (DVE scalar_tensor_tensor + accum)
     against a W tile broadcast via PE (ones outer product); relu(x+b) on
     ACT; special ids (< 4) masked to 0.
  2. eq[p,j,q] = (id[128j+p] == id[128j+q]): ids transposed per column on PE
     (broadcast-transpose into PSUM), one batched [128,8,128] DVE pass.
     gsum[:, j] = eq_j @ twm[:, j] (one PE matmul per column): every token
     gets its within-column duplicate-group sum.
  3. Column j of row r scatters (plain writes, 128 offsets = one per
     partition, a hardware limit) into its OWN table section at offset
     j*VPAD. Within a column, duplicate offsets all carry the identical
     group sum, so collisions are benign; across columns the sections are
     disjoint, so no ordering constraints exist anywhere -> the 32 scatters
     stream back-to-back on GpSimd. The host sums the 8 sections per row
     while unsharding (the standard gather for a sum-sharded output).
Output tables rely on the runtime's zero-donated output buffers
(run_bass_via_pjrt donates np.zeros as the ExternalOutput backing).
"""

import numpy as np

B, S, H, V = 32, 1024, 1024, 250002
N_CORES = 8
B_LOC = B // N_CORES          # 4 batch rows per core
VPAD = 128 * 1954             # 250112 >= V, divisible by 128
N_STILE = S // 128            # 8 column-tiles per row

_compiled = {}


def _build(b_val: float):
    import concourse.bass as bass
    import concourse.tile as tile
    from concourse import bacc, mybir
    from concourse.masks import make_identity

    f32 = mybir.dt.float32
    i32 = mybir.dt.int32
    Alu = mybir.AluOpType

    nc = bacc.Bacc("TRN2", target_bir_lowering=False, debug=False)

    hs = nc.dram_tensor("hs", [B_LOC, S, H], f32, kind="ExternalInput")
    ids = nc.dram_tensor("ids", [B_LOC, S], i32, kind="ExternalInput")
    w = nc.dram_tensor("w", [1, H], f32, kind="ExternalInput")
    tables = [
        [
            nc.dram_tensor(f"t{r}_{j}", [VPAD, 1], f32, kind="ExternalOutput")
            for j in range(N_STILE)
        ]
        for r in range(B_LOC)
    ]

    with tile.TileContext(nc) as tc:
        with (
            tc.tile_pool(name="const", bufs=1) as const_pool,
            tc.tile_pool(name="h", bufs=6) as h_pool,
            tc.tile_pool(name="eq", bufs=2) as eq_pool,
            tc.tile_pool(name="sc", bufs=2) as sc_pool,
            tc.tile_pool(name="sm", bufs=2) as sm_pool,
            tc.tile_pool(name="ps", bufs=2, space="PSUM") as ps_pool,
        ):
            # ---- W broadcast to 128 partitions via PE outer product ----
            ones1 = const_pool.tile([1, 128], f32)
            nc.vector.memset(ones1[:], 1.0)
            w_row = const_pool.tile([1, H], f32)
            nc.sync.dma_start(w_row[:], w[0:1, :])
            wb_ps = ps_pool.tile([128, H], f32, tag="wb_ps", bufs=1)
            for half in range(2):
                sl = slice(512 * half, 512 * (half + 1))
                nc.tensor.matmul(
                    out=wb_ps[:, sl],
                    lhsT=ones1[:],
                    rhs=w_row[:, sl],
                    start=True,
                    stop=True,
                )
            w_bc = const_pool.tile([128, H], f32)
            nc.scalar.copy(w_bc[:], wb_ps[:])

            ident = const_pool.tile([128, 128], f32)
            make_identity(nc, ident[:])

            # ---- all rows' ids in one tile: (p, r, j) = ids[r, 128j+p] ----
            idc_all = sm_pool.tile([128, B_LOC, N_STILE], i32, bufs=1)
            idc_f_all = sm_pool.tile([128, B_LOC, N_STILE], f32, bufs=1)
            for r in range(B_LOC):
                nc.sync.dma_start(
                    idc_all[:, r].rearrange("p (blk k) -> p blk k", k=2),
                    ids[r, :].rearrange("(blk p k) -> p blk k", p=128, k=2),
                )
                nc.scalar.copy(idc_f_all[:, r], idc_all[:, r])

            c3 = [128, N_STILE, 128]
            HALF = N_STILE // 2
            for r in range(B_LOC):
                idc_fr = idc_f_all[:, r]

                def emit_eq(r_, idc_fr_):
                    idT_ps = ps_pool.tile(
                        [128, S], f32, tag="idT_ps", bufs=1, name=f"idT{r_}"
                    )
                    for j in range(N_STILE):
                        nc.tensor.transpose(
                            out=idT_ps[:, 128 * j : 128 * (j + 1)],
                            in_=idc_fr_[:, j : j + 1].to_broadcast([128, 128]),
                            identity=ident[:],
                        )
                    eq8_ = eq_pool.tile(
                        [128, S], f32, tag="eq8", name=f"eq8_{r_}"
                    )
                    nc.vector.tensor_tensor(
                        out=eq8_[:].rearrange("p (j q) -> p j q", j=N_STILE),
                        in0=idc_fr_[:, :, None].to_broadcast(c3),
                        in1=idT_ps[:].rearrange("p (j q) -> p j q", j=N_STILE),
                        op=Alu.is_equal,
                    )
                    return eq8_

                # id-only work: for row 0 it would delay the first matvec
                # (head-of-line on the DVE queue), so emit it after the first
                # half-row instead; later rows emit it up front so the
                # row-tail chain never waits on it.
                eq8 = None
                if r > 0:
                    eq8 = emit_eq(r, idc_fr)

                # ---- matvec + per-half relu/mask/gsum/scatter chains ----
                twraw = sm_pool.tile([128, N_STILE], f32, tag="twraw")
                twrelu = sm_pool.tile([128, N_STILE], f32, tag="twrelu")
                twm = sm_pool.tile([128, N_STILE], f32, tag="twm")
                gsum_ps = ps_pool.tile([128, N_STILE], f32, tag="gsum")
                gsum = sm_pool.tile([128, N_STILE], f32, tag="gsumsb")
                for blk in range(N_STILE // 2):
                    ht = h_pool.tile([128, 2, H], f32, tag="h", bufs=6)
                    nc.sync.dma_start(
                        ht[:],
                        hs[r, 256 * blk : 256 * (blk + 1), :].rearrange(
                            "(p k) h -> p k h", p=128
                        ),
                    )
                    for k in range(2):
                        j = 2 * blk + k
                        prod = sc_pool.tile([128, H], f32, tag="prod")
                        nc.vector.scalar_tensor_tensor(
                            out=prod[:],
                            in0=ht[:, k],
                            scalar=1.0,
                            in1=w_bc[:],
                            op0=Alu.mult,
                            op1=Alu.mult,
                            accum_out=twraw[:, j : j + 1],
                        )
                    if r == 0 and blk == 0:
                        eq8 = emit_eq(r, idc_fr)
                    j = 2 * blk + 1
                    if j % HALF != HALF - 1:
                        continue
                    # half-row [j-HALF+1 .. j] is complete: finish it
                    h0 = j - HALF + 1
                    sl = slice(h0, j + 1)
                    nc.scalar.activation(
                        twrelu[:, sl],
                        twraw[:, sl],
                        mybir.ActivationFunctionType.Relu,
                        bias=float(b_val),
                    )
                    nc.vector.scalar_tensor_tensor(
                        out=twm[:, sl],
                        in0=idc_fr[:, sl],
                        scalar=4.0,
                        in1=twrelu[:, sl],
                        op0=Alu.is_ge,
                        op1=Alu.mult,
                    )
                    for jj in range(h0, j + 1):
                        nc.tensor.matmul(
                            out=gsum_ps[:, jj : jj + 1],
                            lhsT=eq8[:, 128 * jj : 128 * (jj + 1)],
                            rhs=twm[:, jj : jj + 1],
                            start=True,
                            stop=True,
                        )
                        nc.scalar.copy(
                            gsum[:, jj : jj + 1], gsum_ps[:, jj : jj + 1]
                        )
                        nc.gpsimd.indirect_dma_start(
                            out=tables[r][jj][:],
                            out_offset=bass.IndirectOffsetOnAxis(
                                ap=idc_all[:, r, jj : jj + 1], axis=0
                            ),
                            in_=gsum[:, jj : jj + 1],
                            in_offset=None,
                        )

    nc.compile()
    return nc


def _get_nc(b_val: float):
    key = float(b_val)
    if key not in _compiled:
        _compiled[key] = _build(key)
    return _compiled[key]


def kernel(hidden_state, input_ids, W_sparse, b_sparse):
    from concourse.bass_utils import run_bass_kernel_spmd

    hidden_state = np.ascontiguousarray(np.asarray(hidden_state, dtype=np.float32))
    input_ids = np.ascontiguousarray(np.asarray(input_ids, dtype=np.int32))
    W_sparse = np.ascontiguousarray(np.asarray(W_sparse, dtype=np.float32))
    b_val = float(np.asarray(b_sparse).reshape(-1)[0])

    nc = _get_nc(b_val)

    in_maps = []
    for c in range(N_CORES):
        sl = slice(c * B_LOC, (c + 1) * B_LOC)
        in_maps.append(
            {"hs": hidden_state[sl], "ids": input_ids[sl], "w": W_sparse}
        )

    res = run_bass_kernel_spmd(nc, in_maps, list(range(N_CORES)))

    out = np.empty((B, V), dtype=np.float32)
    for c in range(N_CORES):
        for r in range(B_LOC):
            acc = res.results[c][f"t{r}_0"][:V, 0].copy()
            for j in range(1, N_STILE):
                acc += res.results[c][f"t{r}_{j}"][:V, 0]
            out[c * B_LOC + r] = acc
    return out
